# revision 4
# baseline (speedup 1.0000x reference)
"""Multi-head attention on 8 TRN2 NeuronCores.

Reference computation (per batch b):
  q = x @ w_q;  k, v = split(x @ w_kv);  per head: softmax(q k^T / 8) v
  out = ctx @ w_out + b_out

Sharding: core c handles batch b = c // 2 and head-half hh = c % 2
(8 of 16 heads). Per-core work is a perfectly balanced 1/8 of total
MACs. Each core computes a partial out^T (its 8 heads' contribution,
transposed); the host sums the two partials per batch, adds the bias
and transposes back.

Per-core kernel layout (everything transposed, feature-major — this
avoids all on-chip transposes):
  xT  [1024, 2048]  (dl, s)      bf16, host-pretransposed
  QT/KT per head-pair p: [128, 2048] = (2 heads x 64 dh, s)
  V natural [s, dh] with a ones column appended per head -> the
    ctx^T matmul (lhsT = V|1) also produces the softmax denominator
    as psum row 64 for free.
  scores^T tile (s_k=128, q-chunk 1024) = paired K=64 matmuls via PE
    row tiling (two heads concurrently in array rows 0-63 / 64-127)
  P^T = Exp(scale * scores^T) on the scalar engine (psum -> sbuf bf16)
  ctx^T accumulated over 16 k-tiles; normalized by 1/denom which is
    broadcast across partitions with a K=1 ones matmul + DVE recip.
  out^T [1024, 2048] = w_out^T-slices @ ctx^T
"""

import numpy as np
import ml_dtypes

import concourse.bacc as bacc
import concourse.tile as tile
import concourse.mybir as mybir
from concourse.bass_utils import run_bass_kernel_spmd

bf16 = ml_dtypes.bfloat16
FP32 = mybir.dt.float32
BF16 = mybir.dt.bfloat16
EXP = mybir.ActivationFunctionType.Exp

B, S, DL = 4, 2048, 1024
H, DH = 16, 64          # global heads
NH = 8                  # heads per core
HD = NH * DH            # 512 feature cols per core
NPAIR = NH // 2         # 4 head pairs
KT = DL // 128          # 8 k-tiles over d_latent
ST = S // 128           # 16 tiles over sequence
QCH = 1024              # q-chunk (free dim of scores^T tiles)
NQC = S // QCH          # 2 q-chunks
SCALE = 1.0 / np.sqrt(DH)

N_CORES = 8


def _build(reps: int = 1):
    nc = bacc.Bacc(None, target_bir_lowering=False)

    xT = nc.dram_tensor("xT", [DL, S], BF16, kind="ExternalInput")
    wq = nc.dram_tensor("wq", [DL, HD], BF16, kind="ExternalInput")
    wk = nc.dram_tensor("wk", [DL, HD], BF16, kind="ExternalInput")
    wv = nc.dram_tensor("wv", [DL, HD], BF16, kind="ExternalInput")
    wo = nc.dram_tensor("wo", [HD, DL], BF16, kind="ExternalInput")
    out = nc.dram_tensor("out", [DL, S], FP32, kind="ExternalOutput")

    with tile.TileContext(nc) as tc:
        with (
            tc.tile_pool(name="persist", bufs=1) as pp,
            tc.tile_pool(name="pt", bufs=4) as ptp,
            tc.tile_pool(name="small", bufs=4) as smp,
            tc.tile_pool(name="outsb", bufs=2) as osp,
            tc.tile_pool(name="psA", bufs=2, space="PSUM") as psA,
            tc.tile_pool(name="psB", bufs=2, space="PSUM") as psB,
        ):
            for _ in range(reps):
                _body(nc, tc, pp, ptp, smp, osp, psA, psB,
                      xT, wq, wk, wv, wo, out)
    nc.compile()
    return nc


def _body(nc, tc, pp, ptp, smp, osp, psA, psB, xT, wq, wk, wv, wo, out):
    # ---- persistent tiles (tag-keyed; reused across reps) ----
    xt = [pp.tile([128, S], BF16, tag=f"xt{k}", name=f"xt{k}") for k in range(KT)]
    wq_sb = [pp.tile([128, HD], BF16, tag=f"wq{k}", name=f"wq{k}") for k in range(KT)]
    wk_sb = [pp.tile([128, HD], BF16, tag=f"wk{k}", name=f"wk{k}") for k in range(KT)]
    wv_sb = [pp.tile([128, HD], BF16, tag=f"wv{k}", name=f"wv{k}") for k in range(KT)]
    wo_sb = [pp.tile([128, DL], BF16, tag=f"wo{t}", name=f"wo{t}") for t in range(NPAIR)]
    qt = [pp.tile([128, S], BF16, tag=f"qt{p}", name=f"qt{p}") for p in range(NPAIR)]
    kt_ = [pp.tile([128, S], BF16, tag=f"kt{p}", name=f"kt{p}") for p in range(NPAIR)]
    vt = [pp.tile([128, NH * (DH + 1)], BF16, tag=f"vt{m}", name=f"vt{m}") for m in range(ST)]
    ct = [pp.tile([128, S], BF16, tag=f"ct{t}", name=f"ct{t}") for t in range(NPAIR)]
    ones = pp.tile([1, DH], BF16, tag="ones")

    nc.vector.memset(ones[:, :], 1.0)

    # ---- input DMAs ----
    for k in range(KT):
        nc.sync.dma_start(xt[k][:, :], xT[k * 128:(k + 1) * 128, :])
        nc.sync.dma_start(wq_sb[k][:, :], wq[k * 128:(k + 1) * 128, :])
        nc.sync.dma_start(wk_sb[k][:, :], wk[k * 128:(k + 1) * 128, :])
        nc.sync.dma_start(wv_sb[k][:, :], wv[k * 128:(k + 1) * 128, :])
    for t in range(NPAIR):
        nc.sync.dma_start(wo_sb[t][:, :], wo[t * 128:(t + 1) * 128, :])

    # ---- V projection (natural layout [s, hd], ones col per head) ----
    for m in range(ST):
        ps = psA.tile([128, QCH], FP32, tag="sc")
        for k in range(KT):
            nc.tensor.matmul(ps[:, 0:HD],
                             xt[k][:, m * 128:(m + 1) * 128],
                             wv_sb[k][:, :],
                             start=(k == 0), stop=(k == KT - 1))
        v3 = vt[m][:, :].rearrange("p (h c) -> p h c", c=DH + 1)
        nc.vector.tensor_copy(v3[:, :, 0:DH],
                              ps[:, 0:HD].rearrange("p (h c) -> p h c", h=NH))
        nc.vector.memset(v3[:, :, DH:DH + 1], 1.0)

    for p in range(NPAIR):
        # ---- Q^T / K^T projection for this head pair ----
        for dst, w_sb in ((qt[p], wq_sb), (kt_[p], wk_sb)):
            for nch in range(NQC):
                ps = psA.tile([128, QCH], FP32, tag="sc")
                for half in range(2):
                    nsl = slice(half * 512, half * 512 + 512)
                    rsl = slice(nch * QCH + half * 512, nch * QCH + half * 512 + 512)
                    for k in range(KT):
                        nc.tensor.matmul(ps[:, nsl],
                                         w_sb[k][:, p * 128:(p + 1) * 128],
                                         xt[k][:, rsl],
                                         start=(k == 0), stop=(k == KT - 1))
                nc.vector.tensor_copy(dst[:, nch * QCH:(nch + 1) * QCH], ps[:, :])

        # ---- attention for the pair's two heads ----
        for qch in range(NQC):
            qsl = slice(qch * QCH, (qch + 1) * QCH)
            ctx1 = psB.tile([DH + 1, QCH], FP32, tag="ctx")
            ctx2 = psB.tile([DH + 1, QCH], FP32, tag="ctx")
            for ki in range(ST):
                sc1 = psA.tile([128, QCH], FP32, tag="sc")
                sc2 = psA.tile([128, QCH], FP32, tag="sc")
                ksl = slice(ki * 128, (ki + 1) * 128)
                for half in range(2):
                    nsl = slice(half * 512, half * 512 + 512)
                    rsl = slice(qch * QCH + half * 512, qch * QCH + half * 512 + 512)
                    nc.tensor.matmul(sc1[:, nsl], kt_[p][0:64, ksl],
                                     qt[p][0:64, rsl], start=True, stop=True,
                                     tile_position=(0, 0))
                    nc.tensor.matmul(sc2[:, nsl], kt_[p][64:128, ksl],
                                     qt[p][64:128, rsl], start=True, stop=True,
                                     tile_position=(64, 0))
                pt1 = ptp.tile([128, QCH], BF16, tag="pt")
                pt2 = ptp.tile([128, QCH], BF16, tag="pt")
                nc.scalar.activation(pt1[:, :], sc1[:, :], EXP, scale=SCALE)
                nc.scalar.activation(pt2[:, :], sc2[:, :], EXP, scale=SCALE)
                for half in range(2):
                    nsl = slice(half * 512, half * 512 + 512)
                    for hi, (ctx, pt_) in enumerate(((ctx1, pt1), (ctx2, pt2))):
                        lh = 2 * p + hi
                        nc.tensor.matmul(ctx[:, nsl],
                                         vt[ki][:, lh * 65:lh * 65 + 65],
                                         pt_[:, nsl],
                                         start=(ki == 0), stop=(ki == ST - 1))
            # ---- normalize: ct[p][hi*64:(hi+1)*64, qsl] = ctx/denom ----
            for hi, ctx in enumerate((ctx1, ctx2)):
                den = smp.tile([1, QCH], BF16, tag="den")
                nc.vector.tensor_copy(den[:, :], ctx[DH:DH + 1, :])
                bc = psA.tile([128, QCH], FP32, tag="sc")
                for half in range(2):
                    nsl = slice(half * 512, half * 512 + 512)
                    nc.tensor.matmul(bc[0:DH, nsl], ones[:, :], den[:, nsl],
                                     start=True, stop=True)
                rc = smp.tile([DH, QCH], FP32, tag="rc")
                nc.vector.reciprocal(rc[:, :], bc[0:DH, :])
                nc.vector.tensor_mul(ct[p][hi * 64:(hi + 1) * 64, qsl],
                                     ctx[0:DH, :], rc[:, :])

    # ---- out^T projection ----
    for mt in range(KT):
        msl = slice(mt * 128, (mt + 1) * 128)
        ob = osp.tile([128, S], FP32, tag="ob")
        for nch in range(2):
            ps = psA.tile([128, QCH], FP32, tag="sc")
            for half in range(2):
                nsl = slice(half * 512, half * 512 + 512)
                rsl = slice(nch * QCH + half * 512, nch * QCH + half * 512 + 512)
                for t in range(NPAIR):
                    nc.tensor.matmul(ps[:, nsl],
                                     wo_sb[t][:, msl],
                                     ct[t][:, rsl],
                                     start=(t == 0), stop=(t == NPAIR - 1))
            nc.vector.tensor_copy(ob[:, nch * QCH:(nch + 1) * QCH], ps[:, :])
        nc.sync.dma_start(out[msl, :], ob[:, :])


_NC_CACHE = {}


def _get_nc(reps: int = 1):
    if reps not in _NC_CACHE:
        _NC_CACHE[reps] = _build(reps)
    return _NC_CACHE[reps]


def shard_inputs(x, w_q, w_kv, w_out):
    """Full inputs -> per-core in_maps (host-side layout prep)."""
    ins = []
    for c in range(N_CORES):
        b, hh = c // 2, c % 2
        fsl = slice(hh * HD, (hh + 1) * HD)
        ins.append({
            "xT": np.ascontiguousarray(x[b].T).astype(bf16),
            "wq": np.ascontiguousarray(w_q[:, fsl]).astype(bf16),
            "wk": np.ascontiguousarray(w_kv[:, fsl]).astype(bf16),
            "wv": np.ascontiguousarray(w_kv[:, H * DH:][:, fsl]).astype(bf16),
            "wo": np.ascontiguousarray(w_out[fsl, :]).astype(bf16),
        })
    return ins


def unshard_output(results, b_out):
    out = np.empty((B, S, DL), np.float32)
    for b in range(B):
        acc = results[2 * b]["out"] + results[2 * b + 1]["out"]   # [DL, S]
        out[b] = acc.T + b_out
    return out


def kernel(x, w_q, w_kv, w_out, b_out):
    nc = _get_nc()
    ins = shard_inputs(x, w_q, w_kv, w_out)
    res = run_bass_kernel_spmd(nc, ins, core_ids=list(range(N_CORES)))
    return unshard_output(res.results, b_out)


# revision 29
# speedup vs baseline: 347.7048x; 347.7048x over previous
"""Multi-head attention on 8 TRN2 NeuronCores.

Reference computation (per batch b):
  q = x @ w_q;  k, v = split(x @ w_kv);  per head: softmax(q k^T / 8) v
  out = ctx @ w_out + b_out

Sharding: core c handles batch b = c // 2 and head-half hh = c % 2
(8 of 16 heads). Per-core work is a perfectly balanced 1/8 of total
MACs. Each core computes a partial out^T (its 8 heads' contribution,
transposed); the host sums the two partials per batch, adds the bias
and transposes back.

Per-core kernel layout (everything transposed, feature-major — this
avoids all on-chip transposes):
  xT  [1024, 2048]  (dl, s)      bf16, host-pretransposed
  QT/KT per head-pair p: [128, 2048] = (2 heads x 64 dh, s)
  V natural [s, dh] with a ones column appended per head -> the
    ctx^T matmul (lhsT = V|1) also produces the softmax denominator
    as psum row 64 for free.
  scores^T tile (s_k=128, q-chunk 1024) = paired K=64 matmuls via PE
    row tiling (two heads concurrently in array rows 0-63 / 64-127)
  P^T = Exp(scale * scores^T) on the scalar engine (psum -> sbuf bf16)
  ctx^T accumulated over 16 k-tiles; normalized by 1/denom which is
    broadcast across partitions with a K=1 ones matmul + DVE recip.
  out^T [1024, 2048] = w_out^T-slices @ ctx^T
"""

import numpy as np
import ml_dtypes

import concourse.bacc as bacc
import concourse.tile as tile
import concourse.mybir as mybir
from concourse.bass_utils import run_bass_kernel_spmd

bf16 = ml_dtypes.bfloat16
FP32 = mybir.dt.float32
BF16 = mybir.dt.bfloat16
EXP = mybir.ActivationFunctionType.Exp

B, S, DL = 4, 2048, 1024
H, DH = 16, 64          # global heads
NH = 8                  # heads per core
HD = NH * DH            # 512 feature cols per core
NPAIR = NH // 2         # 4 head pairs
KT = DL // 128          # 8 k-tiles over d_latent
ST = S // 128           # 16 tiles over sequence
QCH = 1024              # q-chunk (free dim of scores^T tiles)
NQC = S // QCH          # 2 q-chunks
SCALE = 1.0 / np.sqrt(DH)

N_CORES = 8


def _build(reps: int = 1, loop: int = 0, ablate=()):
    nc = bacc.Bacc(None, target_bir_lowering=False)

    xT = nc.dram_tensor("xT", [DL, S], BF16, kind="ExternalInput")
    wq = nc.dram_tensor("wq", [DL, HD], BF16, kind="ExternalInput")
    wk = nc.dram_tensor("wk", [DL, HD], BF16, kind="ExternalInput")
    wv = nc.dram_tensor("wv", [DL, HD], BF16, kind="ExternalInput")
    wo = nc.dram_tensor("wo", [HD, DL], BF16, kind="ExternalInput")
    out = nc.dram_tensor("out", [DL, S], FP32, kind="ExternalOutput")

    with tile.TileContext(nc) as tc:
        with (
            tc.tile_pool(name="persist", bufs=1) as pp,
            tc.tile_pool(name="pt", bufs=4) as ptp,
            tc.tile_pool(name="small", bufs=2) as smp,
            tc.tile_pool(name="outsb", bufs=2) as osp,
            tc.tile_pool(name="psA", bufs=2, space="PSUM") as psA,
            tc.tile_pool(name="psB", bufs=2, space="PSUM") as psB,
        ):
            if loop:
                with tc.For_i(0, loop, 1):
                    _body(nc, tc, pp, ptp, smp, osp, psA, psB,
                          xT, wq, wk, wv, wo, out, ablate)
            else:
                for _ in range(reps):
                    _body(nc, tc, pp, ptp, smp, osp, psA, psB,
                          xT, wq, wk, wv, wo, out, ablate)
    nc.compile()
    return nc


def _body(nc, tc, pp, ptp, smp, osp, psA, psB, xT, wq, wk, wv, wo, out, ablate=()):
    # ---- persistent tiles (tag-keyed; reused across reps) ----
    xt = [pp.tile([128, S], BF16, tag=f"xt{k}", name=f"xt{k}") for k in range(KT)]
    wq_sb = [pp.tile([128, HD], BF16, tag=f"wq{k}", name=f"wq{k}") for k in range(KT)]
    wk_sb = [pp.tile([128, HD], BF16, tag=f"wk{k}", name=f"wk{k}") for k in range(KT)]
    wv_sb = [pp.tile([128, HD], BF16, tag=f"wv{k}", name=f"wv{k}") for k in range(KT)]
    wo_sb = [pp.tile([128, DL], BF16, tag=f"wo{t}", name=f"wo{t}") for t in range(NPAIR)]
    qt = [pp.tile([128, S], BF16, tag=f"qt{p}", name=f"qt{p}") for p in range(NPAIR)]
    kt_ = [pp.tile([128, S], BF16, tag=f"kt{p}", name=f"kt{p}") for p in range(NPAIR)]
    vt = [pp.tile([128, NH * (DH + 4)], BF16, tag=f"vt{m}", name=f"vt{m}") for m in range(ST)]
    ct = [pp.tile([128, S], BF16, tag=f"ct{t}", name=f"ct{t}") for t in range(NPAIR)]

    # ---- input DMAs ----
    for k in range(KT):
        nc.sync.dma_start(xt[k][:, :], xT[k * 128:(k + 1) * 128, :])
        nc.sync.dma_start(wq_sb[k][:, :], wq[k * 128:(k + 1) * 128, :])
        nc.sync.dma_start(wk_sb[k][:, :], wk[k * 128:(k + 1) * 128, :])
        nc.sync.dma_start(wv_sb[k][:, :], wv[k * 128:(k + 1) * 128, :])
    for t in range(NPAIR):
        nc.sync.dma_start(wo_sb[t][:, :], wo[t * 128:(t + 1) * 128, :])
    if 'proj' in ablate:
        for p_ in range(NPAIR):
            nc.sync.dma_start(qt[p_][:, :], xT[p_ * 128:(p_ + 1) * 128, :])
            nc.sync.dma_start(kt_[p_][:, :], xT[p_ * 128:(p_ + 1) * 128, :])
    if 'vproj' in ablate:
        for m_ in range(ST):
            nc.sync.dma_start(vt[m_][:, :], xT[(m_ % 8) * 128:(m_ % 8) * 128 + 128, 0:NH * (DH + 4)])
    if 'attn' in ablate:
        for t_ in range(NPAIR):
            nc.sync.dma_start(ct[t_][:, :], xT[t_ * 128:(t_ + 1) * 128, :])

    # ---- V projection (natural layout [s, hd], ones col per head) ----
    for m in range(ST if 'vproj' not in ablate else 0):
        ps = psA.tile([128, QCH], FP32, tag="sc")
        for k in range(KT):
            nc.tensor.matmul(ps[:, 0:HD],
                             xt[k][:, m * 128:(m + 1) * 128],
                             wv_sb[k][:, :],
                             start=(k == 0), stop=(k == KT - 1))
        v3 = vt[m][:, :].rearrange("p (h c) -> p h c", c=DH + 4)
        nc.vector.tensor_copy(v3[:, :, 0:DH],
                              ps[:, 0:HD].rearrange("p (h c) -> p h c", h=NH))
        nc.vector.memset(v3[:, :, DH:DH + 4], 1.0)

    # deferred-normalize machinery: at each chunk end the raw ctx and its
    # denominator row are copied to SBUF (releasing psum immediately); the
    # bcast/recip/in-place-mul chain is emitted one chunk later so its
    # cross-engine latency hides under the next chunk's compute.
    norm_pend = []
    BCAST_MASK = [0] * 32

    def flush_norm():
        while norm_pend:
            p_, qsl_, rsrc_ = norm_pend.pop(0)
            rdst = smp.tile([128, QCH], BF16, tag="rdst")
            nc.vector.stream_shuffle(rdst[:, :], rsrc_[:, :], BCAST_MASK)
            for hi_ in range(2):
                psl = slice(hi_ * 64, (hi_ + 1) * 64)
                csl = ct[p_][psl, qsl_]
                nc.vector.tensor_mul(csl, csl,
                                     rdst[psl, hi_ * 512:hi_ * 512 + 512])

    # Q^T/K^T projection emitters, one [128,1024] psum group per call;
    # interleaved into the attention stream of the previous pair.
    def proj_group(p_, dst, w_sb, nch):
        ps = psA.tile([128, QCH], FP32, tag="sc", name="proj")
        for half in range(2):
            nsl = slice(half * 512, half * 512 + 512)
            rsl = slice(nch * QCH + half * 512, nch * QCH + half * 512 + 512)
            for k in range(KT):
                nc.tensor.matmul(ps[:, nsl],
                                 w_sb[k][:, p_ * 128:(p_ + 1) * 128],
                                 xt[k][:, rsl],
                                 start=(k == 0), stop=(k == KT - 1))
        nc.vector.tensor_copy(dst[:, nch * QCH:(nch + 1) * QCH], ps[:, :])

    def proj_groups_for(p_):
        if 'proj' in ablate or p_ >= NPAIR:
            return []
        return [(p_, dst, w_sb, nch)
                for dst, w_sb in ((qt[p_], wq_sb), (kt_[p_], wk_sb))
                for nch in range(NQC)]

    for g in proj_groups_for(0):
        proj_group(*g)

    for p in range(NPAIR):
        pending_proj = proj_groups_for(p + 1)

        # ---- attention for the pair's two heads ----
        # QCH=512 per head; both heads' score tiles share one [128, 1024]
        # psum tile -> ONE exp per k-step covers both heads.
        for qch in range(0 if 'attn' in ablate else 4):
            qsl = slice(qch * 512, (qch + 1) * 512)
            rsrc = smp.tile([128, QCH], BF16, tag="rsrc")
            nc.vector.memset(rsrc[:, :], 1.0)
            if 'ctx' not in ablate:
                ctxp = psB.tile([DH + 4, QCH], FP32, tag="ctx", name="ctxp")
                ctx1 = ctxp[:, 0:512]
                ctx2 = ctxp[:, 512:1024]
            pend = None
            for ki in range(ST):
                sc = psA.tile([128, QCH], FP32, tag="sc", name="sc")
                ksl = slice(ki * 128, (ki + 1) * 128)
                nc.tensor.matmul(sc[:, 0:512], kt_[p][0:64, ksl],
                                 qt[p][0:64, qsl], start=True, stop=True,
                                 tile_position=(0, 0))
                nc.tensor.matmul(sc[:, 512:1024], kt_[p][64:128, ksl],
                                 qt[p][64:128, qsl], start=True, stop=True,
                                 tile_position=(64, 0))
                pt1 = ptp.tile([128, QCH], BF16, tag="pt", name="pt1")
                nc.scalar.activation(pt1[:, :], sc[:, :], EXP, scale=SCALE)

                def emit_ctx(kj, ptj):
                    for hi, ctx in enumerate((ctx1, ctx2)):
                        lh = 2 * p + hi
                        nc.tensor.matmul(ctx[:, :],
                                         vt[kj][:, lh * 68:lh * 68 + 68],
                                         ptj[:, hi * 512:hi * 512 + 512],
                                         start=(kj == 0), stop=(kj == ST - 1))
                if 'ctx' not in ablate:
                    if pend is not None:
                        emit_ctx(*pend)
                    pend = (ki, pt1)
                if ki == 1:
                    flush_norm()
            if 'ctx' not in ablate:
                emit_ctx(*pend)
            # chunk end: raw ctx + denominator row -> SBUF, free psum
            if 'ctx' not in ablate and 'norm' not in ablate:
                for hi, ctx in enumerate((ctx1, ctx2)):
                    with nc.allow_low_precision(reason="softmax denom recip in bf16"):
                        nc.vector.reciprocal(rsrc[0:4, hi * 512:hi * 512 + 512],
                                             ctx[DH:DH + 4, :])
                    nc.vector.tensor_copy(ct[p][hi * 64:(hi + 1) * 64, qsl],
                                          ctx[0:DH, :])
                # replicate the recip rows into every 32-partition quadrant
                # (stream_shuffle only shuffles within quadrants)
                for q_ in (32, 64, 96):
                    nc.sync.dma_start(rsrc[q_:q_ + 4, :], rsrc[0:4, :])
                if 'chain' not in ablate:
                    norm_pend.append((p, qsl, rsrc))
            # interleave one projection group for the next pair
            if pending_proj:
                proj_group(*pending_proj.pop(0))

        for g in pending_proj:
            proj_group(*g)

    flush_norm()

    # ---- out^T projection ----
    for mt in range(KT if 'out' not in ablate else 0):
        msl = slice(mt * 128, (mt + 1) * 128)
        for nch in range(2):
            ob = osp.tile([128, QCH], FP32, tag="ob")
            ps = psA.tile([128, QCH], FP32, tag="sc")
            for half in range(2):
                nsl = slice(half * 512, half * 512 + 512)
                rsl = slice(nch * QCH + half * 512, nch * QCH + half * 512 + 512)
                for t in range(NPAIR):
                    nc.tensor.matmul(ps[:, nsl],
                                     wo_sb[t][:, msl],
                                     ct[t][:, rsl],
                                     start=(t == 0), stop=(t == NPAIR - 1))
            nc.vector.tensor_copy(ob[:, :], ps[:, :])
            nc.sync.dma_start(out[msl, nch * QCH:(nch + 1) * QCH], ob[:, :])


_NC_CACHE = {}


def _get_nc(reps: int = 1):
    if reps not in _NC_CACHE:
        _NC_CACHE[reps] = _build(reps)
    return _NC_CACHE[reps]


def shard_inputs(x, w_q, w_kv, w_out):
    """Full inputs -> per-core in_maps (host-side layout prep)."""
    ins = []
    for c in range(N_CORES):
        b, hh = c // 2, c % 2
        fsl = slice(hh * HD, (hh + 1) * HD)
        ins.append({
            "xT": np.ascontiguousarray(x[b].T).astype(bf16),
            "wq": np.ascontiguousarray(w_q[:, fsl]).astype(bf16),
            "wk": np.ascontiguousarray(w_kv[:, fsl]).astype(bf16),
            "wv": np.ascontiguousarray(w_kv[:, H * DH:][:, fsl]).astype(bf16),
            "wo": np.ascontiguousarray(w_out[fsl, :]).astype(bf16),
        })
    return ins


def unshard_output(results, b_out):
    out = np.empty((B, S, DL), np.float32)
    for b in range(B):
        acc = results[2 * b]["out"] + results[2 * b + 1]["out"]   # [DL, S]
        out[b] = acc.T + b_out
    return out


def kernel(x, w_q, w_kv, w_out, b_out):
    nc = _get_nc()
    ins = shard_inputs(x, w_q, w_kv, w_out)
    res = run_bass_kernel_spmd(nc, ins, core_ids=list(range(N_CORES)))
    return unshard_output(res.results, b_out)
